# revision 2
# baseline (speedup 1.0000x reference)
"""Trainium2 Bass kernel for the DETR-SMPL HMR head (nn_DETRsmpl).

Self-contained: takes the FULL inputs of reference.setup_inputs(), shards
row-wise across 8 NeuronCores (data parallel, weights replicated), runs one
SPMD Bass/Tile program per core, and gathers the full outputs.

Math (reference fp32 semantics, algebraically folded):
    inp_t = [x, theta_t]; a_t = inp_t@W1 + b1; h_t = relu(a_t)
    theta_{t+1} = theta_t + (h_t@W2 + b2)@W3 + b3      (3 iterations)
folds to (W23 = W2@W3, b23 = b2@W3 + b3, Wp = W23@W1theta):
    p_0 = x@W1x                      (PSUM, feature-major, f32r matmuls)
    h_t = relu(p_t + (c0 + t*cp))    (per-partition bias on ScalarE)
    p_{t+1} = p_t + h_t@Wp           (PSUM accumulate in place)
    theta3 = (theta0 + 3*b23) + (h0+h1+h2)@W23
The final matmul uses the activations as the stationary operand so theta3
lands row-major in PSUM, where the rot6d->rotmat Gram-Schmidt runs with
free-dim vector ops spread across VectorE / ScalarE / GpSimdE.
"""

from contextlib import ExitStack

import numpy as np

import concourse.bass as bass
import concourse.tile as tile
from concourse import bacc, mybir
from concourse.bass_utils import run_bass_kernel_spmd

F32 = mybir.dt.float32
F32R = mybir.dt.float32r
AF = mybir.ActivationFunctionType
ALU = mybir.AluOpType
AX = mybir.AxisListType

NPOSE, NBETA, NCAM = 144, 10, 3
NTH = NPOSE + NBETA + NCAM  # 157
NTHP = 160
C = 256
N_CORES = 8

_cache = {}


def _host_prep(inputs, n_cores=N_CORES):
    x = np.asarray(inputs["x"], np.float32)
    W1 = np.asarray(inputs["W1"], np.float64)
    b1 = np.asarray(inputs["b1"], np.float64)
    W2 = np.asarray(inputs["W2"], np.float64)
    b2 = np.asarray(inputs["b2"], np.float64)
    W3 = np.asarray(inputs["W3"], np.float64)
    b3 = np.asarray(inputs["b3"], np.float64)
    theta0 = np.concatenate([
        np.asarray(inputs["init_contrep"], np.float64),
        np.asarray(inputs["init_shape"], np.float64),
        np.asarray(inputs["init_cam"], np.float64),
    ])

    stage, bs, nq, c = x.shape
    n = stage * bs * nq
    assert c == C
    rows = -((-n) // (n_cores * 128)) * 128  # pad shard to 128-row multiple

    W1x = W1[:C]
    W1t = W1[C:]
    W23 = W2 @ W3
    b23 = b2 @ W3 + b3
    Wp = W23 @ W1t
    c0 = theta0 @ W1t + b1
    cp = b23 @ W1t
    tb = theta0 + 3.0 * b23

    w1x32 = np.ascontiguousarray(W1x, np.float32)
    wp32 = np.ascontiguousarray(Wp).astype(np.float32)
    w23p = np.zeros((C, 256), np.float32)
    w23p[:, :NTH] = W23.astype(np.float32)
    biases = np.stack([c0, c0 + cp, c0 + 2 * cp], 0).astype(np.float32)
    btile = np.zeros((128, 6), np.float32)
    for t in range(3):
        for m in range(2):
            btile[:, 2 * t + m] = biases[t, 128 * m:128 * (m + 1)]
    tbp = np.zeros((NTHP,), np.float32)
    tbp[:NTH] = tb.astype(np.float32)

    xf = x.reshape(n, C)
    in_maps = []
    for core in range(n_cores):
        sl = xf[core * rows:min((core + 1) * rows, n)]
        if sl.shape[0] < rows:
            sl = np.concatenate([sl, np.zeros((rows - sl.shape[0], C), np.float32)], 0)
        in_maps.append({
            "xt": np.ascontiguousarray(sl.T),
            "w1x": w1x32, "wp": wp32, "w23": w23p, "btile": btile, "tb": tbp,
        })
    return in_maps, rows, (stage, bs, nq)


def _build(rows, f32r=True, big_tile=1024):
    nc = bacc.Bacc(None, target_bir_lowering=False)

    xt_d = nc.dram_tensor("xt", [C, rows], F32, kind="ExternalInput")
    w1x_d = nc.dram_tensor("w1x", [C, C], F32, kind="ExternalInput")
    wp_d = nc.dram_tensor("wp", [C, C], F32, kind="ExternalInput")
    w23_d = nc.dram_tensor("w23", [C, 256], F32, kind="ExternalInput")
    btile_d = nc.dram_tensor("btile", [128, 6], F32, kind="ExternalInput")
    tb_d = nc.dram_tensor("tb", [NTHP], F32, kind="ExternalInput")
    rot_d = nc.dram_tensor("rot", [rows, 216], F32, kind="ExternalOutput")
    betas_d = nc.dram_tensor("betas", [rows, NBETA], F32, kind="ExternalOutput")
    cam_d = nc.dram_tensor("cam", [rows, NCAM], F32, kind="ExternalOutput")

    mmdt = F32R if f32r else F32
    ldma = (lambda: nc.gpsimd) if f32r else (lambda: nc.sync)  # cast-on-DMA for f32r

    row_tiles = []
    base = 0
    while base < rows:
        nt = min(big_tile, rows - base)
        assert nt % 128 == 0
        row_tiles.append((base, nt))
        base += nt

    with tile.TileContext(nc) as tc, ExitStack() as ctx:
        const = ctx.enter_context(tc.tile_pool(name="const", bufs=1))
        xpool = ctx.enter_context(tc.tile_pool(name="xpool", bufs=2))
        hpool = ctx.enter_context(tc.tile_pool(name="hpool", bufs=2))
        pstate = ctx.enter_context(tc.tile_pool(name="pstate", bufs=1, space="PSUM"))
        ppost = ctx.enter_context(tc.tile_pool(name="ppost", bufs=2, space="PSUM"))
        thpool = ctx.enter_context(tc.tile_pool(name="thpool", bufs=2))
        rpool = ctx.enter_context(tc.tile_pool(name="rpool", bufs=2))
        gs3 = ctx.enter_context(tc.tile_pool(name="gs3", bufs=2))
        gs1 = ctx.enter_context(tc.tile_pool(name="gs1", bufs=2))

        w1x_s = [const.tile([128, C], mmdt, tag=f"w1x{k}", name=f"w1x{k}") for k in range(2)]
        wp_s = [const.tile([128, C], mmdt, tag=f"wp{k}", name=f"wp{k}") for k in range(2)]
        w23_s = [const.tile([128, 256], mmdt, tag=f"w23{k}", name=f"w23{k}") for k in range(2)]
        for k in range(2):
            ldma().dma_start(out=w1x_s[k], in_=w1x_d[128 * k:128 * (k + 1), :])
            ldma().dma_start(out=wp_s[k], in_=wp_d[128 * k:128 * (k + 1), :])
            ldma().dma_start(out=w23_s[k], in_=w23_d[128 * k:128 * (k + 1), :])
        bt_s = const.tile([128, 6], F32, tag="btile", name="bt_s")
        nc.sync.dma_start(out=bt_s, in_=btile_d[:, :])
        tb_s = const.tile([128, NTHP], F32, tag="tb", name="tb_s")
        nc.sync.dma_start(out=tb_s, in_=tb_d.ap().partition_broadcast(128))

        for (base, nt) in row_tiles:
            ns = nt // 128
            nch = (nt + 511) // 512

            xt_s = [xpool.tile([128, nt], mmdt, tag=f"xt{k}", name=f"xt{k}") for k in range(2)]
            for k in range(2):
                ldma().dma_start(out=xt_s[k], in_=xt_d[128 * k:128 * (k + 1), base:base + nt])

            h_s = [[hpool.tile([128, nt], mmdt, tag=f"h{t}{m}", name=f"h{t}{m}")
                    for m in range(2)] for t in range(3)]
            ps = [[pstate.tile([128, 512], F32, tag=f"ps{ch}{m}", name=f"ps{ch}{m}")
                   for m in range(2)] for ch in range(nch)]

            def chw(ch):
                return min(512, nt - 512 * ch)

            # p0 = x @ W1x
            for ch in range(nch):
                w = chw(ch)
                for m in range(2):
                    for k in range(2):
                        nc.tensor.matmul(
                            out=ps[ch][m][:, :w],
                            lhsT=w1x_s[k][:, 128 * m:128 * (m + 1)],
                            rhs=xt_s[k][:, 512 * ch:512 * ch + w],
                            start=(k == 0), stop=False, skip_group_check=True)

            for t in range(3):
                for ch in range(nch):
                    w = chw(ch)
                    for m in range(2):
                        nc.scalar.activation(
                            out=h_s[t][m][:, 512 * ch:512 * ch + w],
                            in_=ps[ch][m][:, :w], func=AF.Relu,
                            bias=bt_s[:, 2 * t + m:2 * t + m + 1], scale=1.0)
                if t < 2:
                    for ch in range(nch):
                        w = chw(ch)
                        for m in range(2):
                            for k in range(2):
                                nc.tensor.matmul(
                                    out=ps[ch][m][:, :w],
                                    lhsT=wp_s[k][:, 128 * m:128 * (m + 1)],
                                    rhs=h_s[t][k][:, 512 * ch:512 * ch + w],
                                    start=False, stop=(t == 1 and k == 1),
                                    skip_group_check=True)

            # hsum01 = h0 + h1 (in place)
            for k in range(2):
                nc.vector.tensor_add(out=h_s[0][k], in0=h_s[0][k], in1=h_s[1][k])

            # theta3 row-major: (hsum01 + h2) @ W23, activations stationary
            th = thpool.tile([128, ns, NTHP], F32, tag="th", name="th")
            for st in range(ns):
                pt = ppost.tile([128, 256], F32, tag="pt", name="pt")
                sl = slice(128 * st, 128 * (st + 1))
                for k in range(2):
                    nc.tensor.matmul(out=pt, lhsT=h_s[0][k][:, sl],
                                     rhs=w23_s[k], start=(k == 0), stop=False)
                for k in range(2):
                    nc.tensor.matmul(out=pt, lhsT=h_s[2][k][:, sl],
                                     rhs=w23_s[k], start=False, stop=(k == 1))
                nc.vector.tensor_add(out=th[:, st, :], in0=pt[:, :NTHP], in1=tb_s)

            # ---- Gram-Schmidt (row-major) ----
            pv = th[:, :, 0:NPOSE].rearrange("p s (j c two) -> p s j c two", c=3, two=2)
            a1 = pv[:, :, :, :, 0]
            a2 = pv[:, :, :, :, 1]

            sq3 = gs3.tile([128, ns, 24, 3], F32, tag="sq3", name="sq3")
            cp3 = gs3.tile([128, ns, 24, 3], F32, tag="cp3", name="cp3")
            u23 = gs3.tile([128, ns, 24, 3], F32, tag="u23", name="u23")
            ux3 = gs3.tile([128, ns, 24, 3], F32, tag="ux3", name="ux3")
            s1 = gs1.tile([128, ns, 24], F32, tag="s1", name="s1")
            cc = gs1.tile([128, ns, 24], F32, tag="cc", name="cc")
            qq = gs1.tile([128, ns, 24], F32, tag="qq", name="qq")
            kk = gs1.tile([128, ns, 24], F32, tag="kk", name="kk")
            s2 = gs1.tile([128, ns, 24], F32, tag="s2", name="s2")
            r1 = gs1.tile([128, ns, 24], F32, tag="r1", name="r1")
            r2 = gs1.tile([128, ns, 24], F32, tag="r2", name="r2")
            gg = gs1.tile([128, ns, 24], F32, tag="gg", name="gg")
            t24 = gs1.tile([128, ns, 24], F32, tag="t24", name="t24")

            def bc(s):
                return s.unsqueeze(3).broadcast_to([128, ns, 24, 3])

            def rsqrt_clamped(dst, src):
                # dst = 1 / max(sqrt(src), 1e-12), reference-faithful eps clamp
                nc.scalar.sqrt(out=dst, in_=src)
                nc.vector.tensor_scalar_max(out=dst, in0=dst, scalar1=1e-12)
                nc.vector.reciprocal_approx_accurate(out=dst, in_=dst, scratch=t24)

            nc.scalar.square(out=sq3, in_=a1)
            nc.vector.reduce_sum(out=s1, in_=sq3, axis=AX.X)
            nc.vector.tensor_mul(out=cp3, in0=a1, in1=a2)
            nc.vector.reduce_sum(out=cc, in_=cp3, axis=AX.X)
            rsqrt_clamped(r1, s1)
            nc.vector.tensor_mul(out=qq, in0=cc, in1=r1)
            nc.vector.tensor_mul(out=kk, in0=qq, in1=r1)
            nc.vector.tensor_mul(out=u23, in0=a1, in1=bc(kk))
            nc.vector.tensor_sub(out=u23, in0=a2, in1=u23)
            nc.vector.tensor_mul(out=sq3, in0=u23, in1=u23)
            nc.vector.reduce_sum(out=s2, in_=sq3, axis=AX.X)
            rsqrt_clamped(r2, s2)
            nc.vector.tensor_mul(out=gg, in0=r1, in1=r2)

            rt = rpool.tile([128, ns, 24, 3, 3], F32, tag="rt", name="rt")
            nc.vector.tensor_mul(out=rt[:, :, :, :, 0], in0=a1, in1=bc(r1))
            nc.vector.tensor_mul(out=rt[:, :, :, :, 1], in0=u23, in1=bc(r2))
            for i in range(3):
                ia, ib = (i + 1) % 3, (i + 2) % 3
                nc.vector.tensor_mul(out=ux3[:, :, :, i],
                                     in0=pv[:, :, :, ia, 0], in1=pv[:, :, :, ib, 1])
                nc.vector.tensor_mul(out=t24,
                                     in0=pv[:, :, :, ib, 0], in1=pv[:, :, :, ia, 1])
                nc.vector.tensor_sub(out=ux3[:, :, :, i], in0=ux3[:, :, :, i], in1=t24)
            nc.vector.tensor_mul(out=rt[:, :, :, :, 2], in0=ux3, in1=bc(gg))

            rot_ap = rot_d[base:base + nt, :].rearrange("(s r) c -> r s c", r=128)
            nc.sync.dma_start(out=rot_ap, in_=rt.rearrange("p s j a b -> p s (j a b)"))
            bet_ap = betas_d[base:base + nt, :].rearrange("(s r) c -> r s c", r=128)
            nc.sync.dma_start(out=bet_ap, in_=th[:, :, NPOSE:NPOSE + NBETA])
            cam_ap = cam_d[base:base + nt, :].rearrange("(s r) c -> r s c", r=128)
            nc.sync.dma_start(out=cam_ap, in_=th[:, :, NPOSE + NBETA:NTH])

    nc.compile()
    return nc


def kernel(x, pred_class, W1, b1, W2, b2, W3, b3, init_contrep, init_shape, init_cam):
    """Full inputs -> (rotmat [s,b,q,24,3,3], betas [s,b,q,10], camera [s,b,q,3])."""
    inputs = dict(x=x, pred_class=pred_class, W1=W1, b1=b1, W2=W2, b2=b2, W3=W3,
                  b3=b3, init_contrep=init_contrep, init_shape=init_shape,
                  init_cam=init_cam)
    in_maps, rows, shape = _host_prep(inputs)
    if rows not in _cache:
        _cache[rows] = _build(rows)
    nc = _cache[rows]
    res = run_bass_kernel_spmd(nc, in_maps, core_ids=list(range(N_CORES)))
    stage, bs, nq = shape
    n = stage * bs * nq
    rot = np.concatenate([r["rot"] for r in res.results], 0)[:n]
    betas = np.concatenate([r["betas"] for r in res.results], 0)[:n]
    cam = np.concatenate([r["cam"] for r in res.results], 0)[:n]
    return (rot.reshape(stage, bs, nq, 24, 3, 3).astype(np.float32),
            betas.reshape(stage, bs, nq, NBETA).astype(np.float32),
            cam.reshape(stage, bs, nq, NCAM).astype(np.float32))
